# revision 105
# baseline (speedup 1.0000x reference)
"""Block-convolution kernel for trn2 (8 NeuronCores, SPMD data-parallel over batch).

Problem: seq_vector [16, 4096, 512] f32, W [7, 512, 512, 7], b [7, 512].
Each block of 8 sequence positions: out position 1+i = conv of kernel size
i+1 (taps 0..i of the block) with weights W[i]; position 0 is zero.

Formulation: one GEMM per output block-slot i:
  Y_i[m, o] = sum_{tap<=i, e} XT[(tap, e), m] * G_i[(tap, e), o] + b[i, o]
with m = (batch, block) flattened. Data-parallel: 2 of 16 batch rows per core.

Device layout (per core):
  XT   [28, 128, 1024]  - X transposed, k-tile major (k = tap*512 + e)
  G    [112, 128, 512]  - masked weights, per-i blocks of (i+1)*4 k-tiles
  BREP [128, 3584]      - bias replicated across partitions
  OUT  [1024, 8, 512]   - per (block-row, position, channel)

Performance structure (graded metric = TimelineSim cost model; ~167us vs a
191us all-bf16 PE floor and 213us for the original f32r version):
  * IO and matmuls in bf16 (same 1 cycle/row PE rate as f32r, half the DMA);
    PSUM accumulates f32; the host upcasts the bf16 output.
  * Each slot computes its leading FP8_KT k-tiles (tap 0 plus half of
    tap 1; 768 contraction elements) in fp8-e4m3 DoubleRow mode (2 k-tiles
    packed per partition -> 0.5 cycles/row, 2x PE rate).  Scales fold into
    the quantized values (xq = x/C8, gq = g*C8) so fp8 products accumulate
    raw next to the bf16 taps in the same PSUM group.  Max-rel error
    measured on HW: 1.74e-2 (gate: 2e-2); the error scales with the
    absolute fp8 term count, so every slot gets the same slice width.
  * A chain of tiny warm-up matmuls keeps the PE busy during the initial DMA
    latency so the p-state ramp happens off the critical path.
  * Per-slot load bundles are prefetched one slot ahead and always emitted
    BEFORE the drain DMAs of the current slot: a drain waits on its DVE add
    and the issuing sequencer blocks head-of-line on that wait.
  * Drains run as per-psum DVE adds into a wide staging tile plus one
    scattered DMA per chunk (HWDGE descriptor-gen at ~625ns/DMA is the
    scarce front resource); the final slot drains per m-tile so the last
    PSUM->SBUF->DRAM chain stays short.  Slot-0 zero writes are deferred to
    mid-kernel where DMA has slack.
"""

import numpy as np
from contextlib import ExitStack

N, S, E = 16, 4096, 512
K = 7           # taps / conv count
BS = 8          # block size
B = S // BS     # 512 blocks per sequence
NCORES = 8
NPC = N // NCORES          # batches per core = 2
M = NPC * B                # 1024 rows per core
KT_TOT = K * (E // 128)    # 28 contraction k-tiles of 128
MT = M // 128              # 8 m-tiles

# Compute dtype for the matmuls: "float32" (exact, 1/4 PE rate) or
# "float32r" / "float16" / "bfloat16" (full PE rate, reduced precision).
MODE = "bfloat16"

NWARM = 64        # warm-up matmuls before the first real one

# Slots >= FP8_FROM compute their tap-0 slice (512 of the contraction) in
# fp8-e4m3 DoubleRow mode: 2x PE rate on that slice.  Error budget: each
# fp8 slice contributes ~1.5e-2 max-rel (vs the 2e-2 gate) regardless of
# slot, because the error grows with the absolute number of fp8 terms while
# the metric divides by the global max.  K (=7) disables fp8 entirely.
FP8_FROM = 0
FP8_KT = 6        # fp8 k-tiles (x128 contraction elems) per slot, capped at
                  # the slot's total; 6 = tap 0 plus half of tap 1
C8 = 3.0          # folded scale: xq = x / C8, gq = g * C8 (products unscaled)
SLOT_ORDER = (0, 1, 2, 3, 4, 5, 6)


def _nf8(i):
    # fp8 k-tiles used by slot i
    return min(FP8_KT, 4 * (i + 1))

_CACHE = {}


def _goff(i):
    # row offset (in 128-row k-tiles) of output-block i inside G
    return 4 * (i * (i + 1) // 2)


def _build_nc(mode):
    import concourse.mybir as mybir
    import concourse.tile as tile
    from concourse import bacc

    mdt = getattr(mybir.dt, mode)
    f32 = mybir.dt.float32
    f8 = mybir.dt.float8e4

    # IO dtype: bf16 end-to-end when the matmul dtype is 2-byte (host upcasts
    # the output); full f32 otherwise.
    iodt = mdt if mybir.dt.np(mdt).itemsize == 2 else f32

    nc = bacc.Bacc("TRN2", target_bir_lowering=False, debug=False)
    xt_d = nc.dram_tensor("xt", [KT_TOT, 128, M], mdt, kind="ExternalInput")
    g_d = nc.dram_tensor("g", [4 * _goff(K - 1) // 4 + 4 * K, 128, E], mdt,
                         kind="ExternalInput")  # [112, 128, 512]
    xq_d = nc.dram_tensor("xq", [FP8_KT, 128, M], f8, kind="ExternalInput")
    gq_d = nc.dram_tensor("gq", [FP8_KT * K, 128, E], f8, kind="ExternalInput")
    br_d = nc.dram_tensor("brep", [128, K * E], iodt, kind="ExternalInput")
    out_d = nc.dram_tensor("out", [M, BS, E], iodt, kind="ExternalOutput")

    with tile.TileContext(nc) as tc, ExitStack() as ctx:
        xt_pool = ctx.enter_context(tc.tile_pool(name="xt", bufs=5))
        g_pool = ctx.enter_context(tc.tile_pool(name="g", bufs=13))
        gq_pool = ctx.enter_context(tc.tile_pool(name="gq", bufs=4))
        bias_pool = ctx.enter_context(tc.tile_pool(name="bias", bufs=3))
        misc_pool = ctx.enter_context(tc.tile_pool(name="misc", bufs=1))
        out_pool = ctx.enter_context(tc.tile_pool(name="out", bufs=4))
        psum_pool = ctx.enter_context(tc.tile_pool(name="ps", bufs=8, space="PSUM"))

        HT = MT // 2  # m-tiles per half-chunk

        def load_xt(tap, nsplit=1):
            t = xt_pool.tile([128, 4 * M], mdt, name="xtt", tag="xtt")
            sp = 4 // nsplit
            for s in range(nsplit):
                nc.sync.dma_start(
                    t[:, s * sp * M : (s + 1) * sp * M].rearrange(
                        "p (s m) -> p s m", s=sp
                    ),
                    xt_d.ap()[4 * tap + s * sp : 4 * tap + (s + 1) * sp, :, :].rearrange(
                        "s p m -> p s m"
                    ),
                )
            return t

        def load_g(i, tap, nsplit=1):
            g_sb = g_pool.tile([128, 4 * E], mdt, name="gsb", tag="gsb")
            kt0 = _goff(i) + 4 * tap
            sp = 4 // nsplit
            for s in range(nsplit):
                nc.sync.dma_start(
                    g_sb[:, s * sp * E : (s + 1) * sp * E].rearrange(
                        "p (s o) -> p s o", s=sp
                    ),
                    g_d.ap()[kt0 + s * sp : kt0 + (s + 1) * sp, :, :].rearrange(
                        "s p o -> p s o"
                    ),
                )
            return g_sb

        def load_gq(i):
            nf = _nf8(i)
            t = gq_pool.tile([128, FP8_KT * E], f8, name="gq", tag="gq")
            nc.sync.dma_start(
                t[:, 0 : nf * E].rearrange("p (s o) -> p s o", s=nf),
                gq_d.ap()[FP8_KT * i : FP8_KT * i + nf, :, :].rearrange(
                    "s p o -> p s o"
                ),
            )
            return t

        def load_g1h(j):
            # bf16 remainder of tap 1 (k-tiles 6..7 of slot j's block)
            t = g_pool.tile([128, 2 * E], mdt, name="g1h", tag="g1h", bufs=3)
            nc.sync.dma_start(
                t[:].rearrange("p (s o) -> p s o", s=2),
                g_d.ap()[_goff(j) + 6 : _goff(j) + 8, :, :].rearrange(
                    "s p o -> p s o"
                ),
            )
            return t

        def load_xt1h():
            # bf16 remainder of tap 1 (k-tiles 2..3 of the tap = xt_d 6..7)
            t = xt_pool.tile([128, 2 * M], mdt, name="xt1h", tag="xt1h", bufs=1)
            nc.sync.dma_start(
                t[:].rearrange("p (s m) -> p s m", s=2),
                xt_d.ap()[6:8, :, :].rearrange("s p m -> p s m"),
            )
            return t

        # --- warm-up: tiny matmuls keep the PE busy + p-state ramping while
        # the first input DMAs are in flight.  They read a memset tile and
        # write a scratch PSUM tile that is recycled by the real groups.
        warm = misc_pool.tile([128, 64], mdt, name="warm", tag="warm")
        nc.vector.memset(warm[:], 0.0)
        wps = psum_pool.tile([128, E], f32, name="wps", tag="ps")
        for _ in range(NWARM):
            nc.tensor.matmul(wps[0:64, 0:64], warm[:], warm[:], start=True, stop=True)

        # zero tile for the slot-0 output positions (written late, when the
        # DMA engines have slack)
        zt = misc_pool.tile([128, 4 * E], iodt, name="zt", tag="zt")
        nc.vector.memset(zt[:], 0.0)

        DR = mybir.MatmulPerfMode.DoubleRow

        def fp8_matmuls(i, psums, m0, mcnt, xq_sb, gq_sb, n_ops, op_base):
            # fp8 DoubleRow slice: _nf8(i)//2 k-tile pairs
            for pair in range(_nf8(i) // 2):
                xv = xq_sb[:, pair * 2 * M : (pair + 1) * 2 * M].rearrange(
                    "p (s m) -> p s m", s=2
                )
                gv = gq_sb[:, pair * 2 * E : (pair + 1) * 2 * E].rearrange(
                    "p (s o) -> p s o", s=2
                )
                for mh in range(mcnt):
                    m = m0 + mh
                    nc.tensor.matmul(
                        psums[mh][:],
                        xv[:, :, m * 128 : (m + 1) * 128],
                        gv,
                        start=(op_base + pair == 0),
                        stop=(op_base + pair == n_ops - 1),
                        perf_mode=DR,
                    )

        # Per-slot resources, filled by prefetch bundles (in consumption order,
        # emitted 1-2 slots ahead of use so loads never queue behind drain
        # DMAs, which block the sequencer head-of-line on their waits).
        xt_sb = [None] * K
        xq_sb = None
        slot_g = [None] * K      # dict tap -> g tile (bf16 taps)
        slot_gq = [None] * K
        slot_bias = [None] * K

        bias_all = misc_pool.tile([128, K * E], iodt, name="bias_all", tag="ball")

        def small_bias(j):
            t = bias_pool.tile([128, E], iodt, name="bias_sb", tag="bias_sb")
            nc.sync.dma_start(t[:], br_d.ap()[:, j * E : (j + 1) * E])
            slot_bias[j] = t

        xq_hi_loaded = [False]  # xq k-tiles 4..5 (used by slots >= 1)

        def prefetch(j):
            """Emit slot j's loads in consumption order."""
            sg = {}
            if j >= 1 and not xq_hi_loaded[0]:
                nc.sync.dma_start(
                    xq_sb[:, 4 * M : 6 * M].rearrange("p (s m) -> p s m", s=2),
                    xq_d.ap()[4:6, :, :].rearrange("s p m -> p s m"),
                )
                xq_hi_loaded[0] = True
            if slot_gq[j] is None:
                slot_gq[j] = load_gq(j)
            elif j == 1:
                # remaining fp8 k-tiles 2..5 of slot 1 (pair 0 was preloaded)
                nf = _nf8(1)
                nc.sync.dma_start(
                    slot_gq[1][:, 2 * E : nf * E].rearrange(
                        "p (s o) -> p s o", s=nf - 2
                    ),
                    gq_d.ap()[FP8_KT + 2 : FP8_KT + nf, :, :].rearrange(
                        "s p o -> p s o"
                    ),
                )
            if j >= 1 and not xq_hi_loaded[0]:
                nc.sync.dma_start(
                    xq_sb[:, 4 * M : 6 * M].rearrange("p (s m) -> p s m", s=2),
                    xq_d.ap()[4:6, :, :].rearrange("s p m -> p s m"),
                )
                xq_hi_loaded[0] = True
            if j >= 1:
                if xt1h[0] is None:
                    xt1h[0] = load_xt1h()
                sg[1] = load_g1h(j)
            if j == 2:
                # combined bias tensor covers slots 2..6 in one DMA
                nc.sync.dma_start(bias_all[:], br_d.ap()[:, :])
                for jj in range(2, K):
                    slot_bias[jj] = bias_all[:, jj * E : (jj + 1) * E]
            elif slot_bias[j] is None:
                small_bias(j)
            for tap in range(2, j + 1):
                sg[tap] = load_g(j, tap, nsplit=1)
            if j >= 2 and xt_sb[j] is None:
                xt_sb[j] = load_xt(j, nsplit=2)
            slot_g[j] = sg

        xt1h = [None]

        first = SLOT_ORDER[0]
        if FP8_FROM == 0 and first == 0:
            # fine-grained startup: slot 0 is pure fp8; gq/xq halves
            # interleaved so the first DoubleRow pair can start earliest
            gq0 = gq_pool.tile([128, FP8_KT * E], f8, name="gq", tag="gq")
            xq_sb = misc_pool.tile([128, FP8_KT * M], f8, name="xq", tag="xq")
            slot_gq[0] = gq0
            nc.sync.dma_start(
                gq0[:, 0 : 4 * E].rearrange("p (s o) -> p s o", s=4),
                gq_d.ap()[0:4, :, :].rearrange("s p o -> p s o"),
            )
            for s in range(2):
                nc.sync.dma_start(
                    xq_sb[:, s * 2 * M : (s + 1) * 2 * M].rearrange(
                        "p (s m) -> p s m", s=2
                    ),
                    xq_d.ap()[2 * s : 2 * s + 2, :, :].rearrange("s p m -> p s m"),
                )
            # bias0 gates slot-0's drain adds (and so the PSUM banks slot 1
            # needs); it goes ahead of slot-1's weights
            small_bias(0)
            # slot 1's first fp8 pair rides in the startup stream so slot 1
            # can begin the moment slot 0's two chunks finish
            gq1_pre = gq_pool.tile([128, FP8_KT * E], f8, name="gq", tag="gq")
            slot_gq[1] = gq1_pre
            nc.sync.dma_start(
                gq1_pre[:, 0 : 2 * E].rearrange("p (s o) -> p s o", s=2),
                gq_d.ap()[FP8_KT : FP8_KT + 2, :, :].rearrange("s p o -> p s o"),
            )
        else:
            small_bias(first)
            prefetch(first)

        for p, i in enumerate(SLOT_ORDER):
            bias_sb = slot_bias[i]
            g_tiles = slot_g[i]
            gq_sb = slot_gq[i]
            npairs = _nf8(i) // 2
            # bf16 work: half of tap 1 (2 k-tiles), then taps 2..i in full
            bf = []  # (xt tile, g tile, n_subs)
            if i >= 1:
                bf.append((xt1h[0], g_tiles[1], 2))
                for tap in range(2, i + 1):
                    bf.append((xt_sb[tap], g_tiles[tap], 4))
            bf_taps = bf  # truthiness used below
            n_ops = npairs + sum(ns for (_, _, ns) in bf)
            if p == 0 and not bf_taps:
                chunks = [(0, HT), (HT, HT)]
            elif p < K - 1:
                chunks = [(0, HT), (HT, HT)]
            else:
                chunks = [(0, 7), (7, 1)]
            for ci, (m0, mcnt) in enumerate(chunks):
                psums = [
                    psum_pool.tile([128, E], f32, name="ps", tag="ps")
                    for _ in range(mcnt)
                ]
                fp8_matmuls(i, psums, m0, mcnt, xq_sb, gq_sb, n_ops, 0)
                op = npairs
                for (xtile, g_sb, n_subs) in bf:
                    for sub in range(n_subs):
                        for mh in range(mcnt):
                            m = m0 + mh
                            nc.tensor.matmul(
                                psums[mh][:],
                                xtile[:, sub * M + m * 128 : sub * M + (m + 1) * 128],
                                g_sb[:, sub * E : (sub + 1) * E],
                                start=(op == 0),
                                stop=(op == n_ops - 1),
                            )
                        op += 1
                if ci == 0:
                    if p + 1 < K:
                        prefetch(SLOT_ORDER[p + 1])
                    if p == 4:
                        # slot-0 zeros: the DMA engines have slack by now and
                        # these depend only on the early memset
                        for h in range(2):
                            nc.sync.dma_start(
                                out_d.ap()[
                                    h * 512 : (h + 1) * 512, 0, :
                                ].rearrange("(s p) o -> p s o", s=4),
                                zt[:].rearrange("p (s o) -> p s o", s=4),
                            )
                # drain: per-psum adds (freeing banks promptly) into one wide
                # staging tile, then a single scattered DMA for the chunk --
                # HWDGE descriptor-gen (~625ns/DMA, serialized) is the scarce
                # front-of-kernel resource.  The last slot drains per m-tile
                # so the final add->DMA chain is not queued behind a wide
                # multi-tile transfer.
                if p == K - 1:
                    for mh in range(mcnt):
                        m = m0 + mh
                        ot = out_pool.tile([128, E], iodt, name="otw")
                        nc.vector.tensor_add(ot[:], psums[mh][:], bias_sb[:])
                        nc.sync.dma_start(
                            out_d.ap()[m * 128 : (m + 1) * 128, i + 1, :], ot[:]
                        )
                else:
                    otw = out_pool.tile([128, mcnt * E], iodt, name="otw")
                    for mh in range(mcnt):
                        nc.vector.tensor_add(
                            otw[:, mh * E : (mh + 1) * E], psums[mh][:], bias_sb[:]
                        )
                    nc.sync.dma_start(
                        out_d.ap()[
                            m0 * 128 : (m0 + mcnt) * 128, i + 1, :
                        ].rearrange("(s p) o -> p s o", s=mcnt),
                        otw[:].rearrange("p (s o) -> p s o", s=mcnt),
                    )

    nc.compile()
    return nc


def _prep_inputs(seq_vector, W, b, mode):
    """Returns (sharded, replicated) input dicts.

    sharded["xt"]: [NCORES*28, 128, 1024] - concat of per-core XT slices.
    replicated["g"], replicated["brep"]: identical on every core.
    """
    np_dt = {"float32": np.float32, "float32r": np.float32,
             "float16": np.float16}.get(mode)
    if np_dt is None:
        import ml_dtypes
        np_dt = ml_dtypes.bfloat16

    import ml_dtypes as _mld
    f8np = _mld.float8_e4m3

    # xt_concat[c, tap*4+sub, p, nl*512 + blk] = seq[2c+nl, blk*8+tap, sub*128+p]
    x6 = seq_vector.reshape(NCORES, NPC, B, BS, E)[:, :, :, :K, :]
    x6 = x6.reshape(NCORES, NPC, B, K, 4, 128)
    xtf = np.ascontiguousarray(
        x6.transpose(0, 3, 4, 5, 1, 2), dtype=np.float32
    )                                                   # [NC, K, 4, 128, NPC, B]
    xt = xtf.astype(np_dt).reshape(NCORES * KT_TOT, 128, M)
    # fp8 X slice: k-tiles 0..FP8_KT-1 (tap 0 + leading subs of tap 1)
    xk = xtf.reshape(NCORES, KT_TOT, 128, M)[:, :FP8_KT]
    xq = np.ascontiguousarray(
        (xk / C8).astype(f8np)
    ).reshape(NCORES * FP8_KT, 128, M)
    gf = np.concatenate(
        [W[i].transpose(2, 1, 0)[: i + 1].reshape((i + 1) * E, E) for i in range(K)],
        axis=0,
    ).astype(np.float32)                                # [14336, 512]
    g = np.ascontiguousarray(gf.astype(np_dt)).reshape(112, 128, E)
    gq_blocks = []
    for i in range(K):
        nf = _nf8(i)
        blk = gf[_goff(i) * 128 : (_goff(i) + nf) * 128] * C8
        if nf < FP8_KT:
            blk = np.concatenate(
                [blk, np.zeros(((FP8_KT - nf) * 128, E), np.float32)], axis=0
            )
        gq_blocks.append(blk)
    gq = np.concatenate(gq_blocks, axis=0).astype(f8np)
    gq = np.ascontiguousarray(gq).reshape(FP8_KT * K, 128, E)
    io_dt = np_dt if np.dtype(np_dt).itemsize == 2 else np.float32
    brep = np.ascontiguousarray(
        np.broadcast_to(b.reshape(1, K * E), (128, K * E)), dtype=io_dt
    )
    return {"xt": xt, "xq": xq}, {"g": g, "gq": gq, "brep": brep}


def _get_runner(mode):
    """Build (once) and return a callable in_maps -> list of per-core out arrays."""
    key = ("runner", mode)
    if key in _CACHE:
        return _CACHE[key]

    import jax
    from jax.sharding import Mesh, PartitionSpec
    from jax.experimental.shard_map import shard_map
    from concourse import bass2jax
    from concourse.bass2jax import _bass_exec_p
    import concourse.mybir as mybir

    nc = _build_nc(mode)
    bass2jax.install_neuronx_cc_hook()

    partition_name = nc.partition_id_tensor.name if nc.partition_id_tensor else None
    in_names, out_names, out_avals, zero_shapes = [], [], [], []
    for alloc in nc.m.functions[0].allocations:
        if not isinstance(alloc, mybir.MemoryLocationSet):
            continue
        name = alloc.memorylocations[0].name
        if alloc.kind == "ExternalInput":
            if name != partition_name:
                in_names.append(name)
        elif alloc.kind == "ExternalOutput":
            out_names.append(name)
            shape = tuple(alloc.tensor_shape)
            dtype = mybir.dt.np(alloc.dtype)
            out_avals.append(jax.core.ShapedArray(shape, dtype))
            zero_shapes.append((shape, dtype))
    n_params = len(in_names)
    n_outs = len(out_avals)
    all_names = list(in_names) + out_names
    if partition_name is not None:
        all_names.append(partition_name)

    def _body(*args):
        operands = list(args)
        if partition_name is not None:
            operands.append(bass2jax.partition_id_tensor())
        outs = _bass_exec_p.bind(
            *operands,
            out_avals=tuple(out_avals),
            in_names=tuple(all_names),
            out_names=tuple(out_names),
            lowering_input_output_aliases=(),
            sim_require_finite=True,
            sim_require_nnan=True,
            nc=nc,
        )
        return tuple(outs)

    devices = jax.devices()[:NCORES]
    mesh = Mesh(np.asarray(devices), ("core",))
    donate = tuple(range(n_params, n_params + n_outs))
    sharded = jax.jit(
        shard_map(
            _body,
            mesh=mesh,
            in_specs=(PartitionSpec("core"),) * (n_params + n_outs),
            out_specs=(PartitionSpec("core"),) * n_outs,
            check_rep=False,
        ),
        donate_argnums=donate,
        keep_unused=True,
    )

    # The kernel writes every element of the output, so the donated
    # "initial output" buffers are pure placeholders. Build them on-device
    # to avoid shipping zero bytes through the tunnel on every call.
    row_sharding = jax.sharding.NamedSharding(mesh, PartitionSpec("core"))

    import jax.numpy as jnp

    _zeros_jit = jax.jit(
        lambda: tuple(
            jnp.zeros((NCORES * s[0], *s[1:]), d) for (s, d) in zero_shapes
        ),
        out_shardings=tuple(row_sharding for _ in zero_shapes),
    )

    def _dev_zeros():
        return list(_zeros_jit())

    def run(sharded_in, replicated_in, timing_iters=0):
        # all inputs concat over cores on axis 0 (replicated ones are tiled)
        in_dev = []
        for name in in_names:
            if name in sharded_in:
                arr = sharded_in[name]
            else:
                r = replicated_in[name]
                arr = np.broadcast_to(
                    r[None], (NCORES, *r.shape)
                ).reshape(NCORES * r.shape[0], *r.shape[1:])
            in_dev.append(jax.device_put(np.ascontiguousarray(arr), row_sharding))
        out_arrs = sharded(*in_dev, *_dev_zeros())
        if timing_iters:
            import time

            for a in out_arrs:
                a.block_until_ready()
            times = []
            for _ in range(timing_iters):
                t0 = time.perf_counter()
                out_arrs = sharded(*in_dev, *out_arrs)
                for a in out_arrs:
                    a.block_until_ready()
                times.append(time.perf_counter() - t0)
            run.last_times = times
        out = np.asarray(out_arrs[0])
        return out.reshape(NCORES, *out_avals[0].shape)

    _CACHE[key] = run
    return run


def kernel(seq_vector, W, b):
    seq_vector = np.asarray(seq_vector, dtype=np.float32)
    W = np.asarray(W, dtype=np.float32)
    b = np.asarray(b, dtype=np.float32)
    run = _get_runner(MODE)
    sharded_in, replicated_in = _prep_inputs(seq_vector, W, b, MODE)
    outs = run(sharded_in, replicated_in)      # [8, 1024, 8, 512]
    return np.ascontiguousarray(outs.reshape(N, S, E), dtype=np.float32)
